# revision 4
# baseline (speedup 1.0000x reference)
"""2D DWT (Haar-family, 2-tap, FFT-reference) Trainium2 kernel.

The reference computes, per (batch, channel) image, an FFT-based circular
convolution with 2-tap filters, zero-padded to 256 and cropped back to
224x224.  Algebraically the whole pipeline reduces to a 2x2 stencil per
output (no wraparound survives the crop):

    row pass:  u[r, c]   = w1 * x[r, c] + w0 * x[r, c+1]   (zero-ext at c=224)
    col pass:  out[r, c] = v1 * u[r, c] + v0 * u[r+1, c]   (zero-ext at r=224)

with (v, w) drawn from {w_l, w_h} x {w_l, w_h} for the four outputs
(ll, lh, hl, hh).

Kernel strategy (per core, 64 of the 512 independent images):
  * the column pass is a matmul with a tiny banded stationary matrix
    S[p, m] = v1*d(p==m) + v0*d(p==m+1)  (contraction over image rows held
    in SBUF partitions),
  * the row pass is folded into PSUM accumulation: two matmuls per output
    tile, the second using a column-shifted view of the same moving data,
    with per-image zero pad columns (stride-225 layout) so a shifted
    450-wide window never mixes adjacent images.

All arithmetic runs on the tensor engine in fp32; scalar/vector engines
only do PSUM->SBUF copies; the kernel is HBM-bandwidth bound.
"""

import sys

for _p in ("/opt/trn_rl_repo", "/root/.axon_site/_ro/trn_rl_repo"):
    if _p not in sys.path:
        sys.path.append(_p)

import numpy as np

import concourse.bass as bass
import concourse.bacc as bacc
import concourse.mybir as mybir
from concourse import tile
from concourse.bass_utils import run_bass_kernel_spmd

N_CORES = 8
IMG = 64          # images per core  (512 total = 8 batch * 64 channels)
H = 224
W = 224
G = 16            # images per supertile
NSG = IMG // G    # supertile image-groups per core
SW = W + 1        # per-image stride in SBUF free dim (1 zero pad col)
FREE = G * SW     # 3600
XCOLS = FREE + 2  # + tail pad (shifted matmul of last window reads col 3600)
SUBS = G // 2     # matmul windows per supertile (2 images each)
N = 2 * SW        # moving free size per matmul (450 <= 512 fp32 limit)
M = 112           # output rows per matmul == half image height


def _build_wmats(w_l: np.ndarray, w_h: np.ndarray) -> np.ndarray:
    """Stationary matrices, laid out [113, 16*112] for a single DMA.

    Slot j = half*8 + o*2 + term holds lhsT for:
      half: row block (0: out rows 0..111 / in rows 0..112,
                       1: out rows 112..223 / in rows 112..223)
      o:    output (0 ll, 1 lh, 2 hl, 3 hh)
      term: 0 -> unshifted moving data (scale v1),
            1 -> column-shifted moving data (scale v0)
    """
    wm = np.zeros((113, 16 * M), np.float32)
    filt = [(w_l, w_l), (w_h, w_l), (w_l, w_h), (w_h, w_h)]  # (col, row) per o
    for half in range(2):
        K = 113 if half == 0 else 112
        for o, (wc, wr) in enumerate(filt):
            S = np.zeros((113, M), np.float32)
            for m in range(M):
                S[m, m] = wc[1]
                if m + 1 < K:
                    S[m + 1, m] = wc[0]
            j = half * 8 + o * 2
            wm[:, j * M:(j + 1) * M] = S * wr[1]
            wm[:, (j + 1) * M:(j + 2) * M] = S * wr[0]
    return wm


def _build_nc() -> bass.Bass:
    nc = bacc.Bacc(
        "TRN2",
        target_bir_lowering=False,
        debug=False,
        num_devices=N_CORES,
    )
    f32 = mybir.dt.float32
    x = nc.dram_tensor("x", [IMG, H, W], f32, kind="ExternalInput")
    wm = nc.dram_tensor("wm", [113, 16 * M], f32, kind="ExternalInput")
    out = nc.dram_tensor("out", [4, IMG, H, W], f32, kind="ExternalOutput")

    with tile.TileContext(nc) as tc:
        with (
            tc.tile_pool(name="wpool", bufs=1) as wpool,
            tc.tile_pool(name="xpool", bufs=3) as xpool,
            tc.tile_pool(name="opool", bufs=2) as opool,
            tc.tile_pool(name="pspool", bufs=2, space="PSUM") as pspool,
        ):
            wt = wpool.tile([113, 16 * M], f32)
            nc.sync.dma_start(out=wt[:, :], in_=wm[:, :])

            for st in range(NSG * 2):
                sg, half = st // 2, st % 2
                K = 113 if half == 0 else 112
                r0 = M * half
                g0 = sg * G

                xt = xpool.tile([113, XCOLS], f32, tag="xt")
                # zero the per-image pad columns + tail columns
                xt_g = xt[:, 0:FREE].rearrange("p (g c) -> p g c", g=G)
                nc.gpsimd.memset(xt_g[:, :, W:SW], 0.0)
                nc.gpsimd.memset(xt[:, FREE:XCOLS], 0.0)
                nc.sync.dma_start(
                    out=xt_g[0:K, :, 0:W],
                    in_=x[g0:g0 + G, r0:r0 + K, :].rearrange("g p c -> p g c"),
                )

                ots = [
                    opool.tile([M, FREE], f32, tag=f"ot{o}", name=f"ot{o}_{st}")
                    for o in range(4)
                ]
                for sub in range(SUBS):
                    s = sub * N
                    for o in range(4):
                        pt = pspool.tile(
                            [M, N], f32, tag=f"ps{o}", name=f"ps{o}_{st}_{sub}"
                        )
                        j0 = (half * 8 + o * 2) * M
                        nc.tensor.matmul(
                            pt[:, :],
                            wt[0:K, j0:j0 + M],
                            xt[0:K, s:s + N],
                            start=True,
                            stop=False,
                        )
                        nc.tensor.matmul(
                            pt[:, :],
                            wt[0:K, j0 + M:j0 + 2 * M],
                            xt[0:K, s + 1:s + 1 + N],
                            start=False,
                            stop=True,
                        )
                        if o < 3:
                            nc.vector.tensor_copy(ots[o][:, s:s + N], pt[:, :])
                        else:
                            nc.scalar.copy(ots[o][:, s:s + N], pt[:, :])

                for o in range(4):
                    nc.sync.dma_start(
                        out=out[o, g0:g0 + G, r0:r0 + M, :].rearrange(
                            "g p c -> p g c"
                        ),
                        in_=ots[o][:, :].rearrange("p (g c) -> p g c", g=G)[
                            :, :, 0:W
                        ],
                    )
    nc.compile()
    return nc


_NC_CACHE: dict = {}


def _get_nc() -> bass.Bass:
    if "nc" not in _NC_CACHE:
        _NC_CACHE["nc"] = _build_nc()
    return _NC_CACHE["nc"]


def kernel(x, w_l, w_h, **run_kwargs):
    x = np.ascontiguousarray(np.asarray(x, dtype=np.float32))
    w_l = np.asarray(w_l, dtype=np.float32).reshape(-1)
    w_h = np.asarray(w_h, dtype=np.float32).reshape(-1)
    assert x.shape == (8, 64, H, W), x.shape
    assert w_l.shape == (2,) and w_h.shape == (2,)

    wm = _build_wmats(w_l, w_h)
    xs = x.reshape(N_CORES * IMG, H, W)
    in_maps = [
        {"x": np.ascontiguousarray(xs[i * IMG:(i + 1) * IMG]), "wm": wm}
        for i in range(N_CORES)
    ]
    res = run_bass_kernel_spmd(
        _get_nc(), in_maps, core_ids=list(range(N_CORES)), **run_kwargs
    )
    full = np.concatenate([r["out"] for r in res.results], axis=1)
    full = full.reshape(4, 8, 64, H, W)
    if run_kwargs:
        kernel.last_result = res  # expose profile info to test harnesses
    return (full[0], full[1], full[2], full[3])


# revision 11
# speedup vs baseline: 2.0816x; 2.0816x over previous
"""2D DWT (2-tap FFT reference) Trainium2 kernel.

The reference's FFT pipeline (pad to 256, circular conv, crop) reduces
algebraically to a 2x2 stencil per output:

    col pass:  v[r, c]   = wc1 * x[r, c] + wc0 * x[r+1, c]   (zero-ext r=224)
    row pass:  out[r, c] = wr1 * v[r, c] + wr0 * v[r, c+1]   (zero-ext c=224)

with (wc, wr) in {w_l, w_h}^2 for the four outputs (ll, lh, hl, hh),
ll = (col l, row l), lh = (col h, row l), hl = (col l, row h),
hh = (col h, row h).

Kernel strategy (per core, 64 of the 512 independent images):
  * column pass on the tensor engine: v = S.T @ X with a banded stationary
    matrix S[p, m] = wc1*d(p==m) + wc0*d(p==m+1); image rows live in SBUF
    partitions, two row-blocks (output rows 0..111 / 112..223) per image.
  * images are packed 16-per-tile along the free dim with stride 225 and a
    zeroed pad column between images, so one 451-wide moving window spans
    two images safely (including the +1 shifted read below).
  * row pass fused into the PSUM->SBUF drain: out = beta*v + gamma*v_sh
    as a single scalar_tensor_tensor op per output when beta is +-1
    (always arrangeable for Haar-type filters by scaling S with wl1),
    else premultiply + stt.
  * output DMAs split across both HWDGE rings (sync + scalar); input DMAs
    use 112-partition transfers (113-partition DMAs degrade to a single
    SDMA engine) plus a 1-partition top-up.
"""

import sys

for _p in ("/opt/trn_rl_repo", "/root/.axon_site/_ro/trn_rl_repo"):
    if _p not in sys.path:
        sys.path.append(_p)

import numpy as np

import concourse.bass as bass
import concourse.bacc as bacc
import concourse.mybir as mybir
from concourse import tile
from concourse.bass_utils import run_bass_kernel_spmd

N_CORES = 8
IMG = 64          # images per core  (512 total = 8 batch * 64 channels)
H = 224
W = 224
G = 16            # images per supertile
NSG = IMG // G    # supertile image-groups per core
SW = W + 1        # per-image stride in SBUF free dim (1 zero pad col)
FREE = G * SW     # 3600
XCOLS = FREE + 2  # + tail pad (last moving window reads col 3600)
SUBS = G // 2     # matmul windows per supertile (2 images each)
N = 2 * SW        # 450 real cols per window; moving width is N+1=451 (<=512)
M = 112           # output rows per matmul == half image height
SPLIT_OUT_RINGS = False  # ACT-ring out-DMAs deadlock at full scale; keep sync
ACT_COPY = True          # v_h PSUM->SBUF bounce on the scalar engine


def _row_coeffs(w_l, w_h):
    """Per-output (beta, gamma) for out = beta*v + gamma*v_sh, after the
    column matrices are scaled by alpha (so that ll/lh get beta == 1)."""
    wl0, wl1 = float(w_l[0]), float(w_l[1])
    wh0, wh1 = float(w_h[0]), float(w_h[1])
    alpha = wl1 if abs(wl1) > 1e-30 else 1.0
    # outputs: 0 ll (col l, row l), 1 lh (col h, row l),
    #          2 hl (col l, row h), 3 hh (col h, row h)
    coeffs = [
        (wl1 / alpha, wl0 / alpha),
        (wl1 / alpha, wl0 / alpha),
        (wh1 / alpha, wh0 / alpha),
        (wh1 / alpha, wh0 / alpha),
    ]
    return alpha, coeffs


def _build_wmats(w_l, w_h):
    """Column-pass stationary matrices scaled by alpha, laid out
    [113, 4*112]: slot j = half*2 + f, f in {0: low, 1: high}."""
    alpha, _ = _row_coeffs(w_l, w_h)
    wm = np.zeros((113, 4 * M), np.float64)
    for half in range(2):
        K = 113 if half == 0 else 112
        for f, wc in enumerate([w_l, w_h]):
            S = np.zeros((113, M), np.float64)
            for m in range(M):
                S[m, m] = float(wc[1]) * alpha
                if m + 1 < K:
                    S[m + 1, m] = float(wc[0]) * alpha
            j = half * 2 + f
            wm[:, j * M:(j + 1) * M] = S
    return wm.astype(np.float32)


def _emit_rowpass(nc, eng, out_ap, sv, beta, gamma, tmp_alloc):
    """out = beta*sv[:, 0:N] + gamma*sv[:, 1:N+1]; sv is an SBUF copy of v."""
    a = mybir.AluOpType
    if abs(beta - 1.0) < 1e-12:
        eng.scalar_tensor_tensor(
            out_ap, sv[:, 1:N + 1], float(gamma), sv[:, 0:N], a.mult, a.add
        )
    elif abs(beta + 1.0) < 1e-12:
        eng.scalar_tensor_tensor(
            out_ap, sv[:, 1:N + 1], float(gamma), sv[:, 0:N], a.mult, a.subtract
        )
    else:  # general: premultiply then stt (2 ops)
        tmp = tmp_alloc()
        eng.tensor_scalar_mul(tmp[:, :], sv[:, 0:N], float(beta))
        eng.scalar_tensor_tensor(
            out_ap, sv[:, 1:N + 1], float(gamma), tmp[:, :], a.mult, a.add
        )


def _build_nc(beta_gamma) -> bass.Bass:
    """beta_gamma: list of 4 (beta, gamma) pairs baked as immediates."""
    nc = bacc.Bacc(
        "TRN2",
        target_bir_lowering=False,
        debug=False,
        num_devices=N_CORES,
    )
    f32 = mybir.dt.float32
    a = mybir.AluOpType
    x = nc.dram_tensor("x", [IMG, H, W], f32, kind="ExternalInput")
    wm = nc.dram_tensor("wm", [113, 4 * M], f32, kind="ExternalInput")
    out = nc.dram_tensor("out", [4, IMG, H, W], f32, kind="ExternalOutput")

    with tile.TileContext(nc) as tc:
        with (
            tc.tile_pool(name="wpool", bufs=1) as wpool,
            tc.tile_pool(name="xpool", bufs=3) as xpool,
            tc.tile_pool(name="opool", bufs=2) as opool,
            tc.tile_pool(name="tpool", bufs=3) as tpool,
            tc.tile_pool(name="pspool", bufs=3, space="PSUM") as pspool,
        ):
            wt = wpool.tile([113, 4 * M], f32)
            nc.sync.dma_start(out=wt[0:112, :], in_=wm[0:112, :])
            nc.sync.dma_start(out=wt[112:113, :], in_=wm[112:113, :])

            for st in range(NSG * 2):
                sg, half = st // 2, st % 2
                K = 113 if half == 0 else 112
                r0 = M * half
                g0 = sg * G

                xt = xpool.tile([113, XCOLS], f32, tag="xt", name=f"xt_{st}")
                xt_g = xt[:, 0:FREE].rearrange("p (g c) -> p g c", g=G)
                nc.gpsimd.memset(xt_g[:, :, W:SW], 0.0)
                nc.gpsimd.memset(xt[:, FREE:XCOLS], 0.0)
                # 112-partition main load + 1-partition top-up (a
                # 113-partition HWDGE DMA collapses onto one SDMA engine)
                nc.sync.dma_start(
                    out=xt_g[0:112, :, 0:W],
                    in_=x[g0:g0 + G, r0:r0 + 112, :].rearrange("g p c -> p g c"),
                )
                if K == 113:
                    nc.sync.dma_start(
                        out=xt_g[112:113, :, 0:W],
                        in_=x[g0:g0 + G, r0 + 112:r0 + 113, :].rearrange(
                            "g p c -> p g c"
                        ),
                    )

                ots = [
                    opool.tile([M, FREE], f32, tag=f"ot{o}", name=f"ot{o}_{st}")
                    for o in range(4)
                ]
                for sub in range(SUBS):
                    s = sub * N
                    svs = []
                    for f in range(2):  # 0: col-low, 1: col-high
                        v = pspool.tile(
                            [M, N + 1], f32, tag=f"v{f}", name=f"v{f}_{st}_{sub}"
                        )
                        j0 = (half * 2 + f) * M
                        nc.tensor.matmul(
                            v[:, :],
                            wt[0:K, j0:j0 + M],
                            xt[0:K, s:s + N + 1],
                            start=True,
                            stop=True,
                        )
                        # bounce to SBUF (walrus: only one PSUM operand per
                        # elementwise instr; SBUF also enables DVE 2x mode)
                        sv = tpool.tile(
                            [M, N + 1], f32, tag=f"sv{f}", name=f"sv{f}_{st}_{sub}"
                        )
                        if f == 0 or not ACT_COPY:
                            nc.vector.tensor_copy(sv[:, :], v[:, :])
                        else:
                            nc.scalar.copy(sv[:, :], v[:, :])
                        svs.append(sv)
                    # outputs 0,2 read v_l; 1,3 read v_h
                    for o, sv in ((0, svs[0]), (1, svs[1]), (2, svs[0]), (3, svs[1])):
                        beta, gamma = beta_gamma[o]

                        def _tmp_alloc(o=o, st=st, sub=sub):
                            return tpool.tile(
                                [M, N], f32, tag="tmp", name=f"tmp{o}_{st}_{sub}"
                            )

                        _emit_rowpass(
                            nc, nc.vector, ots[o][:, s:s + N], sv, beta, gamma,
                            _tmp_alloc,
                        )

                for o in range(4):
                    eng = nc.sync if (o < 2 or not SPLIT_OUT_RINGS) else nc.scalar
                    eng.dma_start(
                        out=out[o, g0:g0 + G, r0:r0 + M, :].rearrange(
                            "g p c -> p g c"
                        ),
                        in_=ots[o][:, :].rearrange("p (g c) -> p g c", g=G)[
                            :, :, 0:W
                        ],
                    )
    nc.compile()
    return nc


_NC_CACHE: dict = {}


def _get_nc(w_l, w_h) -> bass.Bass:
    _, coeffs = _row_coeffs(w_l, w_h)
    key = tuple(coeffs[o] for o in range(4))
    if key not in _NC_CACHE:
        _NC_CACHE[key] = _build_nc(coeffs)
    return _NC_CACHE[key]


def kernel(x, w_l, w_h, **run_kwargs):
    x = np.ascontiguousarray(np.asarray(x, dtype=np.float32))
    w_l = np.asarray(w_l, dtype=np.float32).reshape(-1)
    w_h = np.asarray(w_h, dtype=np.float32).reshape(-1)
    assert x.shape == (8, 64, H, W), x.shape
    assert w_l.shape == (2,) and w_h.shape == (2,)

    wm = _build_wmats(w_l, w_h)
    xs = x.reshape(N_CORES * IMG, H, W)
    in_maps = [
        {"x": np.ascontiguousarray(xs[i * IMG:(i + 1) * IMG]), "wm": wm}
        for i in range(N_CORES)
    ]
    res = run_bass_kernel_spmd(
        _get_nc(w_l, w_h), in_maps, core_ids=list(range(N_CORES)), **run_kwargs
    )
    full = np.concatenate([r["out"] for r in res.results], axis=1)
    full = full.reshape(4, 8, 64, H, W)
    if run_kwargs:
        kernel.last_result = res  # expose profile info to test harnesses
    return (full[0], full[1], full[2], full[3])


# revision 13
# speedup vs baseline: 2.1834x; 1.0489x over previous
"""2D DWT (2-tap FFT reference) Trainium2 kernel.

The reference's FFT pipeline (pad to 256, circular conv, crop) reduces
algebraically to a 2x2 stencil per output:

    col pass:  v[r, c]   = wc1 * x[r, c] + wc0 * x[r+1, c]   (zero-ext r=224)
    row pass:  out[r, c] = wr1 * v[r, c] + wr0 * v[r, c+1]   (zero-ext c=224)

with (wc, wr) in {w_l, w_h}^2 for the four outputs (ll, lh, hl, hh),
ll = (col l, row l), lh = (col h, row l), hl = (col l, row h),
hh = (col h, row h).

Kernel strategy (per core, 64 of the 512 independent images):
  * column pass on the tensor engine: v = S.T @ X with a banded stationary
    matrix S[p, m] = wc1*d(p==m) + wc0*d(p==m+1); image rows live in SBUF
    partitions, two row-blocks (output rows 0..111 / 112..223) per image.
  * images are packed 16-per-tile along the free dim with stride 225 and a
    zeroed pad column between images, so one 451-wide moving window spans
    two images safely (including the +1 shifted read below).
  * row pass fused into the PSUM->SBUF drain: out = beta*v + gamma*v_sh
    as a single scalar_tensor_tensor op per output when beta is +-1
    (always arrangeable for Haar-type filters by scaling S with wl1),
    else premultiply + stt.
  * output DMAs split across both HWDGE rings (sync + scalar); input DMAs
    use 112-partition transfers (113-partition DMAs degrade to a single
    SDMA engine) plus a 1-partition top-up.
"""

import sys

for _p in ("/opt/trn_rl_repo", "/root/.axon_site/_ro/trn_rl_repo"):
    if _p not in sys.path:
        sys.path.append(_p)

import numpy as np

import concourse.bass as bass
import concourse.bacc as bacc
import concourse.mybir as mybir
from concourse import tile
from concourse.bass_utils import run_bass_kernel_spmd

N_CORES = 8
IMG = 64          # images per core  (512 total = 8 batch * 64 channels)
H = 224
W = 224
G = 16            # images per supertile
NSG = IMG // G    # supertile image-groups per core
SW = W + 1        # per-image stride in SBUF free dim (1 zero pad col)
FREE = G * SW     # 3600
XCOLS = FREE + 2  # + tail pad (last moving window reads col 3600)
SUBS = G // 2     # matmul windows per supertile (2 images each)
N = 2 * SW        # 450 real cols per window; moving width is N+1=451 (<=512)
M = 112           # output rows per matmul == half image height
SPLIT_OUT_RINGS = False  # ACT-ring out-DMAs deadlock at full scale; keep sync
ACT_COPY = True          # v_h PSUM->SBUF bounce on the scalar engine
IN_ON_GPSIMD = True      # input DMAs via SWDGE to overlap with sync-ring outs


def _row_coeffs(w_l, w_h):
    """Per-output (beta, gamma) for out = beta*v + gamma*v_sh, after the
    column matrices are scaled by alpha (so that ll/lh get beta == 1)."""
    wl0, wl1 = float(w_l[0]), float(w_l[1])
    wh0, wh1 = float(w_h[0]), float(w_h[1])
    alpha = wl1 if abs(wl1) > 1e-30 else 1.0
    # outputs: 0 ll (col l, row l), 1 lh (col h, row l),
    #          2 hl (col l, row h), 3 hh (col h, row h)
    coeffs = [
        (wl1 / alpha, wl0 / alpha),
        (wl1 / alpha, wl0 / alpha),
        (wh1 / alpha, wh0 / alpha),
        (wh1 / alpha, wh0 / alpha),
    ]
    return alpha, coeffs


def _build_wmats(w_l, w_h):
    """Column-pass stationary matrices scaled by alpha, laid out
    [113, 4*112]: slot j = half*2 + f, f in {0: low, 1: high}."""
    alpha, _ = _row_coeffs(w_l, w_h)
    wm = np.zeros((113, 4 * M), np.float64)
    for half in range(2):
        K = 113 if half == 0 else 112
        for f, wc in enumerate([w_l, w_h]):
            S = np.zeros((113, M), np.float64)
            for m in range(M):
                S[m, m] = float(wc[1]) * alpha
                if m + 1 < K:
                    S[m + 1, m] = float(wc[0]) * alpha
            j = half * 2 + f
            wm[:, j * M:(j + 1) * M] = S
    return wm.astype(np.float32)


def _emit_rowpass(nc, eng, out_ap, sv, beta, gamma, tmp_alloc):
    """out = beta*sv[:, 0:N] + gamma*sv[:, 1:N+1]; sv is an SBUF copy of v."""
    a = mybir.AluOpType
    if abs(beta - 1.0) < 1e-12:
        eng.scalar_tensor_tensor(
            out_ap, sv[:, 1:N + 1], float(gamma), sv[:, 0:N], a.mult, a.add
        )
    elif abs(beta + 1.0) < 1e-12:
        eng.scalar_tensor_tensor(
            out_ap, sv[:, 1:N + 1], float(gamma), sv[:, 0:N], a.mult, a.subtract
        )
    else:  # general: premultiply then stt (2 ops)
        tmp = tmp_alloc()
        eng.tensor_scalar_mul(tmp[:, :], sv[:, 0:N], float(beta))
        eng.scalar_tensor_tensor(
            out_ap, sv[:, 1:N + 1], float(gamma), tmp[:, :], a.mult, a.add
        )


def _build_nc(beta_gamma) -> bass.Bass:
    """beta_gamma: list of 4 (beta, gamma) pairs baked as immediates."""
    nc = bacc.Bacc(
        "TRN2",
        target_bir_lowering=False,
        debug=False,
        num_devices=N_CORES,
    )
    f32 = mybir.dt.float32
    a = mybir.AluOpType
    x = nc.dram_tensor("x", [IMG, H, W], f32, kind="ExternalInput")
    wm = nc.dram_tensor("wm", [113, 4 * M], f32, kind="ExternalInput")
    out = nc.dram_tensor("out", [4, IMG, H, W], f32, kind="ExternalOutput")

    with tile.TileContext(nc) as tc:
        with (
            tc.tile_pool(name="wpool", bufs=1) as wpool,
            tc.tile_pool(name="xpool", bufs=3) as xpool,
            tc.tile_pool(name="opool", bufs=2) as opool,
            tc.tile_pool(name="tpool", bufs=3) as tpool,
            tc.tile_pool(name="pspool", bufs=3, space="PSUM") as pspool,
        ):
            wt = wpool.tile([113, 4 * M], f32)
            nc.sync.dma_start(out=wt[0:112, :], in_=wm[0:112, :])
            nc.sync.dma_start(out=wt[112:113, :], in_=wm[112:113, :])

            for st in range(NSG * 2):
                sg, half = st // 2, st % 2
                K = 113 if half == 0 else 112
                r0 = M * half
                g0 = sg * G

                xt = xpool.tile([113, XCOLS], f32, tag="xt", name=f"xt_{st}")
                xt_g = xt[:, 0:FREE].rearrange("p (g c) -> p g c", g=G)
                nc.gpsimd.memset(xt_g[:, :, W:SW], 0.0)
                nc.gpsimd.memset(xt[:, FREE:XCOLS], 0.0)
                # 112-partition main load + 1-partition top-up (a
                # 113-partition HWDGE DMA collapses onto one SDMA engine)
                in_eng = nc.gpsimd if IN_ON_GPSIMD else nc.sync
                in_eng.dma_start(
                    out=xt_g[0:112, :, 0:W],
                    in_=x[g0:g0 + G, r0:r0 + 112, :].rearrange("g p c -> p g c"),
                )
                if K == 113:
                    in_eng.dma_start(
                        out=xt_g[112:113, :, 0:W],
                        in_=x[g0:g0 + G, r0 + 112:r0 + 113, :].rearrange(
                            "g p c -> p g c"
                        ),
                    )

                ots = [
                    opool.tile([M, FREE], f32, tag=f"ot{o}", name=f"ot{o}_{st}")
                    for o in range(4)
                ]
                for sub in range(SUBS):
                    s = sub * N
                    svs = []
                    for f in range(2):  # 0: col-low, 1: col-high
                        v = pspool.tile(
                            [M, N + 1], f32, tag=f"v{f}", name=f"v{f}_{st}_{sub}"
                        )
                        j0 = (half * 2 + f) * M
                        nc.tensor.matmul(
                            v[:, :],
                            wt[0:K, j0:j0 + M],
                            xt[0:K, s:s + N + 1],
                            start=True,
                            stop=True,
                        )
                        # bounce to SBUF (walrus: only one PSUM operand per
                        # elementwise instr; SBUF also enables DVE 2x mode)
                        sv = tpool.tile(
                            [M, N + 1], f32, tag=f"sv{f}", name=f"sv{f}_{st}_{sub}"
                        )
                        if f == 0 or not ACT_COPY:
                            nc.vector.tensor_copy(sv[:, :], v[:, :])
                        else:
                            nc.scalar.copy(sv[:, :], v[:, :])
                        svs.append(sv)
                    # outputs 0,2 read v_l; 1,3 read v_h
                    for o, sv in ((0, svs[0]), (1, svs[1]), (2, svs[0]), (3, svs[1])):
                        beta, gamma = beta_gamma[o]

                        def _tmp_alloc(o=o, st=st, sub=sub):
                            return tpool.tile(
                                [M, N], f32, tag="tmp", name=f"tmp{o}_{st}_{sub}"
                            )

                        _emit_rowpass(
                            nc, nc.vector, ots[o][:, s:s + N], sv, beta, gamma,
                            _tmp_alloc,
                        )

                for o in range(4):
                    eng = nc.sync if (o < 2 or not SPLIT_OUT_RINGS) else nc.scalar
                    eng.dma_start(
                        out=out[o, g0:g0 + G, r0:r0 + M, :].rearrange(
                            "g p c -> p g c"
                        ),
                        in_=ots[o][:, :].rearrange("p (g c) -> p g c", g=G)[
                            :, :, 0:W
                        ],
                    )
    nc.compile()
    return nc


_NC_CACHE: dict = {}


def _get_nc(w_l, w_h) -> bass.Bass:
    _, coeffs = _row_coeffs(w_l, w_h)
    key = tuple(coeffs[o] for o in range(4))
    if key not in _NC_CACHE:
        _NC_CACHE[key] = _build_nc(coeffs)
    return _NC_CACHE[key]


def kernel(x, w_l, w_h, **run_kwargs):
    x = np.ascontiguousarray(np.asarray(x, dtype=np.float32))
    w_l = np.asarray(w_l, dtype=np.float32).reshape(-1)
    w_h = np.asarray(w_h, dtype=np.float32).reshape(-1)
    assert x.shape == (8, 64, H, W), x.shape
    assert w_l.shape == (2,) and w_h.shape == (2,)

    wm = _build_wmats(w_l, w_h)
    xs = x.reshape(N_CORES * IMG, H, W)
    in_maps = [
        {"x": np.ascontiguousarray(xs[i * IMG:(i + 1) * IMG]), "wm": wm}
        for i in range(N_CORES)
    ]
    res = run_bass_kernel_spmd(
        _get_nc(w_l, w_h), in_maps, core_ids=list(range(N_CORES)), **run_kwargs
    )
    full = np.concatenate([r["out"] for r in res.results], axis=1)
    full = full.reshape(4, 8, 64, H, W)
    if run_kwargs:
        kernel.last_result = res  # expose profile info to test harnesses
    return (full[0], full[1], full[2], full[3])


# revision 16
# speedup vs baseline: 2.4777x; 1.1348x over previous
"""2D DWT (2-tap FFT reference) Trainium2 kernel.

The reference's FFT pipeline (pad to 256, circular conv, crop) reduces
algebraically to a 2x2 stencil per output:

    col pass:  v[r, c]   = wc1 * x[r, c] + wc0 * x[r+1, c]   (zero-ext r=224)
    row pass:  out[r, c] = wr1 * v[r, c] + wr0 * v[r, c+1]   (zero-ext c=224)

with (wc, wr) in {w_l, w_h}^2 for the four outputs: ll = (col l, row l),
lh = (col h, row l), hl = (col l, row h), hh = (col h, row h).

Kernel strategy (per core, 64 of the 512 independent images):
  * column pass on the tensor engine: v = S.T @ X with a banded stationary
    matrix S[p, m] = wc1*d(p==m) + wc0*d(p==m+1); image rows in SBUF
    partitions, two 112-row blocks per image; 16 images packed contiguously
    along the free dim (no pad columns), matmul windows of 2 images (448).
  * row pass fused into the PSUM drain: bounce v to SBUF (scalar engine),
    then out = beta*v + gamma*v_shift as one scalar_tensor_tensor per
    output over a [112, 2, 223] view (per-image, so no cross-image leak),
    plus a tiny per-image boundary column op (out[:,223] = beta*v[:,223]).
    beta is +-1 for Haar-type filters (S is pre-scaled by wl1); otherwise
    a premultiply fallback is used.
  * all DRAM tensors use a custom per-core layout [.., half, p, g, c] so
    every DMA descriptor is one fully contiguous 14336-byte run per
    partition (the host pre/post-transposes in numpy); input DMAs go via
    the gpsimd SWDGE queue, output DMAs via the sync HWDGE ring, so they
    overlap.
"""

import sys

for _p in ("/opt/trn_rl_repo", "/root/.axon_site/_ro/trn_rl_repo"):
    if _p not in sys.path:
        sys.path.append(_p)

import numpy as np

import concourse.bass as bass
import concourse.bacc as bacc
import concourse.mybir as mybir
from concourse import tile
from concourse.bass_utils import run_bass_kernel_spmd

N_CORES = 8
IMG = 64          # images per core  (512 total = 8 batch * 64 channels)
H = 224
W = 224
G = 16            # images per supertile
NSG = IMG // G    # supertile image-groups per core
FREE = G * W      # 3584 (contiguous, no pads)
SUBS = G // 2     # matmul windows per supertile (2 images each)
N = 2 * W         # 448 moving cols per window (<=512 fp32 limit)
M = 112           # output rows per matmul == half image height


def _row_coeffs(w_l, w_h):
    """Per-output (beta, gamma) for out = beta*v + gamma*v_sh, after the
    column matrices are scaled by alpha (so ll/lh get beta == 1)."""
    wl0, wl1 = float(w_l[0]), float(w_l[1])
    wh0, wh1 = float(w_h[0]), float(w_h[1])
    alpha = wl1 if abs(wl1) > 1e-30 else 1.0
    coeffs = [
        (wl1 / alpha, wl0 / alpha),   # ll: col l, row l
        (wl1 / alpha, wl0 / alpha),   # lh: col h, row l
        (wh1 / alpha, wh0 / alpha),   # hl: col l, row h
        (wh1 / alpha, wh0 / alpha),   # hh: col h, row h
    ]
    return alpha, coeffs


def _build_wmats(w_l, w_h):
    """Column-pass stationary matrices scaled by alpha, laid out
    [113, 4*112]: slot j = half*2 + f, f in {0: low, 1: high}."""
    alpha, _ = _row_coeffs(w_l, w_h)
    wm = np.zeros((113, 4 * M), np.float64)
    for half in range(2):
        K = 113 if half == 0 else 112
        for f, wc in enumerate([w_l, w_h]):
            S = np.zeros((113, M), np.float64)
            for m in range(M):
                S[m, m] = float(wc[1]) * alpha
                if m + 1 < K:
                    S[m + 1, m] = float(wc[0]) * alpha
            j = half * 2 + f
            wm[:, j * M:(j + 1) * M] = S
    return wm.astype(np.float32)


def _build_nc(beta_gamma) -> bass.Bass:
    """beta_gamma: list of 4 (beta, gamma) pairs baked as immediates."""
    nc = bacc.Bacc(
        "TRN2",
        target_bir_lowering=False,
        debug=False,
        num_devices=N_CORES,
    )
    f32 = mybir.dt.float32
    a = mybir.AluOpType
    # custom layouts: one contiguous (g, c) run per partition per DMA
    x = nc.dram_tensor("x", [NSG, 2, M, G, W], f32, kind="ExternalInput")
    wm = nc.dram_tensor("wm", [113, 4 * M], f32, kind="ExternalInput")
    out = nc.dram_tensor("out", [4, NSG, 2, M, G, W], f32, kind="ExternalOutput")

    with tile.TileContext(nc) as tc:
        with (
            tc.tile_pool(name="wpool", bufs=1) as wpool,
            tc.tile_pool(name="xpool", bufs=3) as xpool,
            tc.tile_pool(name="opool", bufs=2) as opool,
            tc.tile_pool(name="tpool", bufs=3) as tpool,
            tc.tile_pool(name="pspool", bufs=3, space="PSUM") as pspool,
        ):
            wt = wpool.tile([113, 4 * M], f32)
            nc.gpsimd.dma_start(out=wt[0:112, :], in_=wm[0:112, :])
            nc.gpsimd.dma_start(out=wt[112:113, :], in_=wm[112:113, :])

            for st in range(NSG * 2):
                sg, half = st // 2, st % 2
                K = 113 if half == 0 else 112

                xt = xpool.tile([113, FREE], f32, tag="xt", name=f"xt_{st}")
                nc.gpsimd.dma_start(
                    out=xt[0:112, :],
                    in_=x[sg, half].rearrange("p g c -> p (g c)"),
                )
                if K == 113:
                    # row 112 of this block == row 0 of the next half-block
                    nc.gpsimd.dma_start(
                        out=xt[112:113, :],
                        in_=x[sg, 1, 0:1].rearrange("p g c -> p (g c)"),
                    )

                ots = [
                    opool.tile([M, FREE], f32, tag=f"ot{o}", name=f"ot{o}_{st}")
                    for o in range(4)
                ]
                for sub in range(SUBS):
                    s = sub * N
                    svs = []
                    for f in range(2):  # 0: col-low, 1: col-high
                        v = pspool.tile(
                            [M, N], f32, tag=f"v{f}", name=f"v{f}_{st}_{sub}"
                        )
                        j0 = (half * 2 + f) * M
                        nc.tensor.matmul(
                            v[:, :],
                            wt[0:K, j0:j0 + M],
                            xt[0:K, s:s + N],
                            start=True,
                            stop=True,
                        )
                        # bounce to SBUF (one PSUM operand max per elemwise op)
                        sv = tpool.tile(
                            [M, N], f32, tag=f"sv{f}", name=f"sv{f}_{st}_{sub}"
                        )
                        nc.scalar.copy(sv[:, :], v[:, :])
                        svs.append(sv)
                    # row pass: per-image views so the shift never crosses
                    # an image boundary; boundary col 223 handled separately
                    for o, sv in ((0, svs[0]), (1, svs[1]), (2, svs[0]), (3, svs[1])):
                        beta, gamma = beta_gamma[o]
                        sv2 = sv[:, :].rearrange("p (i c) -> p i c", i=2)
                        ot2 = ots[o][:, s:s + N].rearrange("p (i c) -> p i c", i=2)
                        plain = abs(gamma - 1.0) < 1e-12  # out = v_sh +- v
                        # gpsimd (Pool) only runs plain TensorTensor, not
                        # TensorScalarPtr; give it output 3 when eligible
                        eng = nc.gpsimd if (o == 3 and plain) else nc.vector
                        if plain and abs(beta - 1.0) < 1e-12:
                            eng.tensor_add(
                                ot2[:, :, 0:W - 1], sv2[:, :, 1:W],
                                sv2[:, :, 0:W - 1],
                            )
                        elif plain and abs(beta + 1.0) < 1e-12:
                            eng.tensor_sub(
                                ot2[:, :, 0:W - 1], sv2[:, :, 1:W],
                                sv2[:, :, 0:W - 1],
                            )
                        elif abs(beta - 1.0) < 1e-12:
                            eng.scalar_tensor_tensor(
                                ot2[:, :, 0:W - 1], sv2[:, :, 1:W], float(gamma),
                                sv2[:, :, 0:W - 1], a.mult, a.add,
                            )
                        elif abs(beta + 1.0) < 1e-12:
                            eng.scalar_tensor_tensor(
                                ot2[:, :, 0:W - 1], sv2[:, :, 1:W], float(gamma),
                                sv2[:, :, 0:W - 1], a.mult, a.subtract,
                            )
                        else:
                            tmp = tpool.tile(
                                [M, N], f32, tag="tmp", name=f"tmp{o}_{st}_{sub}"
                            )
                            tmp2 = tmp[:, :].rearrange("p (i c) -> p i c", i=2)
                            eng.tensor_scalar_mul(
                                tmp2[:, :, 0:W - 1], sv2[:, :, 0:W - 1], float(beta)
                            )
                            eng.scalar_tensor_tensor(
                                ot2[:, :, 0:W - 1], sv2[:, :, 1:W], float(gamma),
                                tmp2[:, :, 0:W - 1], a.mult, a.add,
                            )
                        # boundary column: out[:, 223] = beta * v[:, 223]
                        nc.vector.tensor_scalar_mul(
                            ot2[:, :, W - 1:W], sv2[:, :, W - 1:W], float(beta)
                        )

                for o in range(4):
                    nc.sync.dma_start(
                        out=out[o, sg, half].rearrange("p g c -> p (g c)"),
                        in_=ots[o][:, :],
                    )
    nc.compile()
    return nc


_NC_CACHE: dict = {}


def _get_nc(w_l, w_h) -> bass.Bass:
    _, coeffs = _row_coeffs(w_l, w_h)
    key = tuple(coeffs[o] for o in range(4))
    if key not in _NC_CACHE:
        _NC_CACHE[key] = _build_nc(coeffs)
    return _NC_CACHE[key]


def kernel(x, w_l, w_h, **run_kwargs):
    x = np.asarray(x, dtype=np.float32)
    w_l = np.asarray(w_l, dtype=np.float32).reshape(-1)
    w_h = np.asarray(w_h, dtype=np.float32).reshape(-1)
    assert x.shape == (8, 64, H, W), x.shape
    assert w_l.shape == (2,) and w_h.shape == (2,)

    wm = _build_wmats(w_l, w_h)
    # per-core relayout: (IMG, 224, 224) -> (NSG, half, p, g, c)
    xs = x.reshape(N_CORES, NSG, G, 2, M, W).transpose(0, 1, 3, 4, 2, 5)
    in_maps = [
        {"x": np.ascontiguousarray(xs[i]), "wm": wm} for i in range(N_CORES)
    ]
    res = run_bass_kernel_spmd(
        _get_nc(w_l, w_h), in_maps, core_ids=list(range(N_CORES)), **run_kwargs
    )
    # gather + inverse relayout: [4, NSG, half, p, g, c] -> [4, IMG, H, W]
    full = np.stack([r["out"] for r in res.results], axis=1)
    # full: [4, core, NSG, 2, M, G, W] -> [4, core, NSG, G, 2, M, W]
    full = full.transpose(0, 1, 2, 5, 3, 4, 6).reshape(4, 8, 64, H, W)
    if run_kwargs:
        kernel.last_result = res  # expose profile info to test harnesses
    return (full[0], full[1], full[2], full[3])


# revision 18
# speedup vs baseline: 2.5432x; 1.0264x over previous
"""2D DWT (2-tap FFT reference) Trainium2 kernel.

The reference's FFT pipeline (pad to 256, circular conv, crop) reduces
algebraically to a 2x2 stencil per output:

    col pass:  v[r, c]   = wc1 * x[r, c] + wc0 * x[r+1, c]   (zero-ext r=224)
    row pass:  out[r, c] = wr1 * v[r, c] + wr0 * v[r, c+1]   (zero-ext c=224)

with (wc, wr) in {w_l, w_h}^2 for the four outputs: ll = (col l, row l),
lh = (col h, row l), hl = (col l, row h), hh = (col h, row h).

Kernel strategy (per core, 64 of the 512 independent images):
  * column pass on the tensor engine: v = S.T @ X with a banded stationary
    matrix S[p, m] = wc1*d(p==m) + wc0*d(p==m+1); image rows in SBUF
    partitions, two 112-row blocks per image; 16 images packed contiguously
    along the free dim (no pad columns), matmul windows of 2 images (448).
  * row pass fused into the PSUM drain: bounce v to SBUF (scalar engine),
    then out = beta*v + gamma*v_shift as one scalar_tensor_tensor per
    output over a [112, 2, 223] view (per-image, so no cross-image leak),
    plus a tiny per-image boundary column op (out[:,223] = beta*v[:,223]).
    beta is +-1 for Haar-type filters (S is pre-scaled by wl1); otherwise
    a premultiply fallback is used.
  * all DRAM tensors use a custom per-core layout [.., half, p, g, c] so
    every DMA descriptor is one fully contiguous 14336-byte run per
    partition (the host pre/post-transposes in numpy); input DMAs go via
    the gpsimd SWDGE queue, output DMAs via the sync HWDGE ring, so they
    overlap.
"""

import sys

for _p in ("/opt/trn_rl_repo", "/root/.axon_site/_ro/trn_rl_repo"):
    if _p not in sys.path:
        sys.path.append(_p)

import numpy as np

import concourse.bass as bass
import concourse.bacc as bacc
import concourse.mybir as mybir
from concourse import tile
from concourse.bass_utils import run_bass_kernel_spmd

N_CORES = 8
IMG = 64          # images per core  (512 total = 8 batch * 64 channels)
H = 224
W = 224
G = 16            # images per supertile
NSG = IMG // G    # supertile image-groups per core
FREE = G * W      # 3584 (contiguous, no pads)
SUBS = G // 2     # matmul windows per supertile (2 images each)
N = 2 * W         # 448 moving cols per window (<=512 fp32 limit)
M = 112           # output rows per matmul == half image height


def _row_coeffs(w_l, w_h):
    """Per-output (beta, gamma) for out = beta*v + gamma*v_sh, after the
    column matrices are scaled by alpha (so ll/lh get beta == 1)."""
    wl0, wl1 = float(w_l[0]), float(w_l[1])
    wh0, wh1 = float(w_h[0]), float(w_h[1])
    alpha = wl1 if abs(wl1) > 1e-30 else 1.0
    coeffs = [
        (wl1 / alpha, wl0 / alpha),   # ll: col l, row l
        (wl1 / alpha, wl0 / alpha),   # lh: col h, row l
        (wh1 / alpha, wh0 / alpha),   # hl: col l, row h
        (wh1 / alpha, wh0 / alpha),   # hh: col h, row h
    ]
    return alpha, coeffs


def _build_wmats(w_l, w_h):
    """Column-pass stationary matrices scaled by alpha, laid out
    [113, 4*112]: slot j = half*2 + f, f in {0: low, 1: high}."""
    alpha, _ = _row_coeffs(w_l, w_h)
    wm = np.zeros((113, 4 * M), np.float64)
    for half in range(2):
        K = 113 if half == 0 else 112
        for f, wc in enumerate([w_l, w_h]):
            S = np.zeros((113, M), np.float64)
            for m in range(M):
                S[m, m] = float(wc[1]) * alpha
                if m + 1 < K:
                    S[m + 1, m] = float(wc[0]) * alpha
            j = half * 2 + f
            wm[:, j * M:(j + 1) * M] = S
    return wm.astype(np.float32)


def _build_nc(beta_gamma) -> bass.Bass:
    """beta_gamma: list of 4 (beta, gamma) pairs baked as immediates."""
    nc = bacc.Bacc(
        "TRN2",
        target_bir_lowering=False,
        debug=False,
        num_devices=N_CORES,
    )
    f32 = mybir.dt.float32
    a = mybir.AluOpType
    # custom layouts: one contiguous (g, c) run per partition per DMA
    x = nc.dram_tensor("x", [NSG, 2, M, G, W], f32, kind="ExternalInput")
    wm = nc.dram_tensor("wm", [113, 4 * M], f32, kind="ExternalInput")
    out = nc.dram_tensor("out", [4, NSG, 2, M, G, W], f32, kind="ExternalOutput")

    with tile.TileContext(nc) as tc:
        with (
            tc.tile_pool(name="wpool", bufs=1) as wpool,
            tc.tile_pool(name="xpool", bufs=3) as xpool,
            tc.tile_pool(name="opool", bufs=2) as opool,
            tc.tile_pool(name="tpool", bufs=3) as tpool,
            tc.tile_pool(name="pspool", bufs=3, space="PSUM") as pspool,
        ):
            wt = wpool.tile([113, 4 * M], f32)
            nc.gpsimd.dma_start(out=wt[0:112, :], in_=wm[0:112, :])
            nc.gpsimd.dma_start(out=wt[112:113, :], in_=wm[112:113, :])

            # persistent SBUF bounce buffers with a zero pad column per
            # image (stride 225) so the row-pass shift reads zero at the
            # image boundary; pads are zeroed once, manually rotated x3
            NBUF = 3
            svbufs = []
            for f in range(2):
                row = []
                for k in range(NBUF):
                    b = wpool.tile(
                        [M, 2 * (W + 1)], f32, tag=f"svb{f}_{k}",
                        name=f"svb{f}_{k}",
                    )
                    nc.gpsimd.memset(
                        b[:, :].rearrange("p (i c) -> p i c", i=2)[:, :, W:W + 1],
                        0.0,
                    )
                    row.append(b)
                svbufs.append(row)

            for st in range(NSG * 2):
                sg, half = st // 2, st % 2
                K = 113 if half == 0 else 112

                xt = xpool.tile([113, FREE], f32, tag="xt", name=f"xt_{st}")
                nc.gpsimd.dma_start(
                    out=xt[0:112, :],
                    in_=x[sg, half].rearrange("p g c -> p (g c)"),
                )
                if K == 113:
                    # row 112 of this block == row 0 of the next half-block
                    nc.gpsimd.dma_start(
                        out=xt[112:113, :],
                        in_=x[sg, 1, 0:1].rearrange("p g c -> p (g c)"),
                    )

                ots = [
                    opool.tile([M, FREE], f32, tag=f"ot{o}", name=f"ot{o}_{st}")
                    for o in range(4)
                ]
                for sub in range(SUBS):
                    s = sub * N
                    widx = st * SUBS + sub
                    svs = []
                    for f in range(2):  # 0: col-low, 1: col-high
                        v = pspool.tile(
                            [M, N], f32, tag=f"v{f}", name=f"v{f}_{st}_{sub}"
                        )
                        j0 = (half * 2 + f) * M
                        nc.tensor.matmul(
                            v[:, :],
                            wt[0:K, j0:j0 + M],
                            xt[0:K, s:s + N],
                            start=True,
                            stop=True,
                        )
                        # bounce to SBUF (one PSUM operand max per elemwise
                        # op); write only real cols, pads stay zero
                        sv = svbufs[f][widx % NBUF]
                        sv2 = sv[:, :].rearrange("p (i c) -> p i c", i=2)
                        nc.scalar.copy(
                            sv2[:, :, 0:W],
                            v[:, :].rearrange("p (i c) -> p i c", i=2),
                        )
                        svs.append(sv2)
                    # row pass over full 224-col views; the shifted read
                    # hits the zero pad at each image boundary
                    for o, sv2 in ((0, svs[0]), (1, svs[1]), (2, svs[0]), (3, svs[1])):
                        beta, gamma = beta_gamma[o]
                        ot2 = ots[o][:, s:s + N].rearrange("p (i c) -> p i c", i=2)
                        sh = sv2[:, :, 1:W + 1]
                        base = sv2[:, :, 0:W]
                        plain = abs(gamma - 1.0) < 1e-12  # out = v_sh +- v
                        # gpsimd (Pool) only runs plain TensorTensor, not
                        # TensorScalarPtr; give it output 3 when eligible
                        eng = nc.gpsimd if (o == 3 and plain) else nc.vector
                        if plain and abs(beta - 1.0) < 1e-12:
                            eng.tensor_add(ot2[:, :, :], sh, base)
                        elif plain and abs(beta + 1.0) < 1e-12:
                            eng.tensor_sub(ot2[:, :, :], sh, base)
                        elif abs(beta - 1.0) < 1e-12:
                            eng.scalar_tensor_tensor(
                                ot2[:, :, :], sh, float(gamma), base,
                                a.mult, a.add,
                            )
                        elif abs(beta + 1.0) < 1e-12:
                            eng.scalar_tensor_tensor(
                                ot2[:, :, :], sh, float(gamma), base,
                                a.mult, a.subtract,
                            )
                        else:
                            tmp = tpool.tile(
                                [M, N], f32, tag="tmp", name=f"tmp{o}_{st}_{sub}"
                            )
                            tmp2 = tmp[:, :].rearrange("p (i c) -> p i c", i=2)
                            eng.tensor_scalar_mul(tmp2[:, :, :], base, float(beta))
                            eng.scalar_tensor_tensor(
                                ot2[:, :, :], sh, float(gamma), tmp2[:, :, :],
                                a.mult, a.add,
                            )

                for o in range(4):
                    nc.sync.dma_start(
                        out=out[o, sg, half].rearrange("p g c -> p (g c)"),
                        in_=ots[o][:, :],
                    )
    nc.compile()
    return nc


_NC_CACHE: dict = {}


def _get_nc(w_l, w_h) -> bass.Bass:
    _, coeffs = _row_coeffs(w_l, w_h)
    key = tuple(coeffs[o] for o in range(4))
    if key not in _NC_CACHE:
        _NC_CACHE[key] = _build_nc(coeffs)
    return _NC_CACHE[key]


def kernel(x, w_l, w_h, **run_kwargs):
    x = np.asarray(x, dtype=np.float32)
    w_l = np.asarray(w_l, dtype=np.float32).reshape(-1)
    w_h = np.asarray(w_h, dtype=np.float32).reshape(-1)
    assert x.shape == (8, 64, H, W), x.shape
    assert w_l.shape == (2,) and w_h.shape == (2,)

    wm = _build_wmats(w_l, w_h)
    # per-core relayout: (IMG, 224, 224) -> (NSG, half, p, g, c)
    xs = x.reshape(N_CORES, NSG, G, 2, M, W).transpose(0, 1, 3, 4, 2, 5)
    in_maps = [
        {"x": np.ascontiguousarray(xs[i]), "wm": wm} for i in range(N_CORES)
    ]
    res = run_bass_kernel_spmd(
        _get_nc(w_l, w_h), in_maps, core_ids=list(range(N_CORES)), **run_kwargs
    )
    # gather + inverse relayout: [4, NSG, half, p, g, c] -> [4, IMG, H, W]
    full = np.stack([r["out"] for r in res.results], axis=1)
    # full: [4, core, NSG, 2, M, G, W] -> [4, core, NSG, G, 2, M, W]
    full = full.transpose(0, 1, 2, 5, 3, 4, 6).reshape(4, 8, 64, H, W)
    if run_kwargs:
        kernel.last_result = res  # expose profile info to test harnesses
    return (full[0], full[1], full[2], full[3])


# revision 20
# speedup vs baseline: 2.5935x; 1.0198x over previous
"""2D DWT (2-tap FFT reference) Trainium2 kernel.

The reference's FFT pipeline (pad to 256, circular conv, crop) reduces
algebraically to a 2x2 stencil per output:

    col pass:  v[r, c]   = wc1 * x[r, c] + wc0 * x[r+1, c]   (zero-ext r=224)
    row pass:  out[r, c] = wr1 * v[r, c] + wr0 * v[r, c+1]   (zero-ext c=224)

with (wc, wr) in {w_l, w_h}^2 for the four outputs: ll = (col l, row l),
lh = (col h, row l), hl = (col l, row h), hh = (col h, row h).

Kernel strategy (per core, 64 of the 512 independent images):
  * column pass on the tensor engine: v = S.T @ X with a banded stationary
    matrix S[p, m] = wc1*d(p==m) + wc0*d(p==m+1); image rows in SBUF
    partitions, two 112-row blocks per image; 16 images packed contiguously
    along the free dim (no pad columns), matmul windows of 2 images (448).
  * row pass fused into the PSUM drain: bounce v to SBUF (scalar engine),
    then out = beta*v + gamma*v_shift as one scalar_tensor_tensor per
    output over a [112, 2, 223] view (per-image, so no cross-image leak),
    plus a tiny per-image boundary column op (out[:,223] = beta*v[:,223]).
    beta is +-1 for Haar-type filters (S is pre-scaled by wl1); otherwise
    a premultiply fallback is used.
  * all DRAM tensors use a custom per-core layout [.., half, p, g, c] so
    every DMA descriptor is one fully contiguous 14336-byte run per
    partition (the host pre/post-transposes in numpy); input DMAs go via
    the gpsimd SWDGE queue, output DMAs via the sync HWDGE ring, so they
    overlap.
"""

import sys

for _p in ("/opt/trn_rl_repo", "/root/.axon_site/_ro/trn_rl_repo"):
    if _p not in sys.path:
        sys.path.append(_p)

import numpy as np

import concourse.bass as bass
import concourse.bacc as bacc
import concourse.mybir as mybir
from concourse import tile
from concourse.bass_utils import run_bass_kernel_spmd

N_CORES = 8
IMG = 64          # images per core  (512 total = 8 batch * 64 channels)
H = 224
W = 224
G = 8             # images per supertile
NSG = IMG // G    # supertile image-groups per core
FREE = G * W      # 1792 (contiguous, no pads)
SUBS = G // 2     # matmul windows per supertile (2 images each)
N = 2 * W         # 448 moving cols per window (<=512 fp32 limit)
M = 112           # output rows per matmul == half image height


def _row_coeffs(w_l, w_h):
    """Per-output (beta, gamma) for out = beta*v + gamma*v_sh, after the
    column matrices are scaled by alpha (so ll/lh get beta == 1)."""
    wl0, wl1 = float(w_l[0]), float(w_l[1])
    wh0, wh1 = float(w_h[0]), float(w_h[1])
    alpha = wl1 if abs(wl1) > 1e-30 else 1.0
    coeffs = [
        (wl1 / alpha, wl0 / alpha),   # ll: col l, row l
        (wl1 / alpha, wl0 / alpha),   # lh: col h, row l
        (wh1 / alpha, wh0 / alpha),   # hl: col l, row h
        (wh1 / alpha, wh0 / alpha),   # hh: col h, row h
    ]
    return alpha, coeffs


def _build_wmats(w_l, w_h):
    """Column-pass stationary matrices scaled by alpha, laid out
    [113, 4*112]: slot j = half*2 + f, f in {0: low, 1: high}."""
    alpha, _ = _row_coeffs(w_l, w_h)
    wm = np.zeros((113, 4 * M), np.float64)
    for half in range(2):
        K = 113 if half == 0 else 112
        for f, wc in enumerate([w_l, w_h]):
            S = np.zeros((113, M), np.float64)
            for m in range(M):
                S[m, m] = float(wc[1]) * alpha
                if m + 1 < K:
                    S[m + 1, m] = float(wc[0]) * alpha
            j = half * 2 + f
            wm[:, j * M:(j + 1) * M] = S
    return wm.astype(np.float32)


def _build_nc(beta_gamma) -> bass.Bass:
    """beta_gamma: list of 4 (beta, gamma) pairs baked as immediates."""
    nc = bacc.Bacc(
        "TRN2",
        target_bir_lowering=False,
        debug=False,
        num_devices=N_CORES,
    )
    f32 = mybir.dt.float32
    a = mybir.AluOpType
    # custom layouts: one contiguous (g, c) run per partition per DMA
    x = nc.dram_tensor("x", [NSG, 2, M, G, W], f32, kind="ExternalInput")
    wm = nc.dram_tensor("wm", [113, 4 * M], f32, kind="ExternalInput")
    out = nc.dram_tensor("out", [NSG, 2, M, 4, G, W], f32, kind="ExternalOutput")

    with tile.TileContext(nc) as tc:
        with (
            tc.tile_pool(name="wpool", bufs=1) as wpool,
            tc.tile_pool(name="xpool", bufs=4) as xpool,
            tc.tile_pool(name="opool", bufs=3) as opool,
            tc.tile_pool(name="tpool", bufs=3) as tpool,
            tc.tile_pool(name="pspool", bufs=3, space="PSUM") as pspool,
        ):
            wt = wpool.tile([113, 4 * M], f32)
            nc.gpsimd.dma_start(out=wt[0:112, :], in_=wm[0:112, :])
            nc.gpsimd.dma_start(out=wt[112:113, :], in_=wm[112:113, :])

            # persistent SBUF bounce buffers with a zero pad column per
            # image (stride 225) so the row-pass shift reads zero at the
            # image boundary; pads are zeroed once, manually rotated x3
            NBUF = 3
            svbufs = []
            for f in range(2):
                row = []
                for k in range(NBUF):
                    b = wpool.tile(
                        [M, 2 * (W + 1)], f32, tag=f"svb{f}_{k}",
                        name=f"svb{f}_{k}",
                    )
                    nc.gpsimd.memset(
                        b[:, :].rearrange("p (i c) -> p i c", i=2)[:, :, W:W + 1],
                        0.0,
                    )
                    row.append(b)
                svbufs.append(row)

            for st in range(NSG * 2):
                sg, half = st // 2, st % 2
                K = 113 if half == 0 else 112

                xt = xpool.tile([113, FREE], f32, tag="xt", name=f"xt_{st}")
                nc.gpsimd.dma_start(
                    out=xt[0:112, :],
                    in_=x[sg, half].rearrange("p g c -> p (g c)"),
                )
                if K == 113:
                    # row 112 of this block == row 0 of the next half-block
                    nc.gpsimd.dma_start(
                        out=xt[112:113, :],
                        in_=x[sg, 1, 0:1].rearrange("p g c -> p (g c)"),
                    )

                otall = opool.tile(
                    [M, 4 * FREE], f32, tag="otall", name=f"otall_{st}"
                )
                for sub in range(SUBS):
                    s = sub * N
                    widx = st * SUBS + sub
                    svs = []
                    for f in range(2):  # 0: col-low, 1: col-high
                        v = pspool.tile(
                            [M, N], f32, tag=f"v{f}", name=f"v{f}_{st}_{sub}"
                        )
                        j0 = (half * 2 + f) * M
                        nc.tensor.matmul(
                            v[:, :],
                            wt[0:K, j0:j0 + M],
                            xt[0:K, s:s + N],
                            start=True,
                            stop=True,
                        )
                        # bounce to SBUF (one PSUM operand max per elemwise
                        # op); write only real cols, pads stay zero
                        sv = svbufs[f][widx % NBUF]
                        sv2 = sv[:, :].rearrange("p (i c) -> p i c", i=2)
                        nc.scalar.copy(
                            sv2[:, :, 0:W],
                            v[:, :].rearrange("p (i c) -> p i c", i=2),
                        )
                        svs.append(sv2)
                    # row pass over full 224-col views; the shifted read
                    # hits the zero pad at each image boundary
                    for o, sv2 in ((0, svs[0]), (1, svs[1]), (2, svs[0]), (3, svs[1])):
                        beta, gamma = beta_gamma[o]
                        ob = o * FREE + s
                        ot2 = otall[:, ob:ob + N].rearrange("p (i c) -> p i c", i=2)
                        sh = sv2[:, :, 1:W + 1]
                        base = sv2[:, :, 0:W]
                        plain = abs(gamma - 1.0) < 1e-12  # out = v_sh +- v
                        # gpsimd (Pool) only runs plain TensorTensor, not
                        # TensorScalarPtr; give it output 3 when eligible
                        eng = nc.gpsimd if (o == 3 and plain) else nc.vector
                        if plain and abs(beta - 1.0) < 1e-12:
                            eng.tensor_add(ot2[:, :, :], sh, base)
                        elif plain and abs(beta + 1.0) < 1e-12:
                            eng.tensor_sub(ot2[:, :, :], sh, base)
                        elif abs(beta - 1.0) < 1e-12:
                            eng.scalar_tensor_tensor(
                                ot2[:, :, :], sh, float(gamma), base,
                                a.mult, a.add,
                            )
                        elif abs(beta + 1.0) < 1e-12:
                            eng.scalar_tensor_tensor(
                                ot2[:, :, :], sh, float(gamma), base,
                                a.mult, a.subtract,
                            )
                        else:
                            tmp = tpool.tile(
                                [M, N], f32, tag="tmp", name=f"tmp{o}_{st}_{sub}"
                            )
                            tmp2 = tmp[:, :].rearrange("p (i c) -> p i c", i=2)
                            eng.tensor_scalar_mul(tmp2[:, :, :], base, float(beta))
                            eng.scalar_tensor_tensor(
                                ot2[:, :, :], sh, float(gamma), tmp2[:, :, :],
                                a.mult, a.add,
                            )

                nc.sync.dma_start(
                    out=out[sg, half].rearrange("p o g c -> p (o g c)"),
                    in_=otall[:, :],
                )
    nc.compile()
    return nc


_NC_CACHE: dict = {}


def _get_nc(w_l, w_h) -> bass.Bass:
    _, coeffs = _row_coeffs(w_l, w_h)
    key = tuple(coeffs[o] for o in range(4))
    if key not in _NC_CACHE:
        _NC_CACHE[key] = _build_nc(coeffs)
    return _NC_CACHE[key]


def kernel(x, w_l, w_h, **run_kwargs):
    x = np.asarray(x, dtype=np.float32)
    w_l = np.asarray(w_l, dtype=np.float32).reshape(-1)
    w_h = np.asarray(w_h, dtype=np.float32).reshape(-1)
    assert x.shape == (8, 64, H, W), x.shape
    assert w_l.shape == (2,) and w_h.shape == (2,)

    wm = _build_wmats(w_l, w_h)
    # per-core relayout: (IMG, 224, 224) -> (NSG, half, p, g, c)
    xs = x.reshape(N_CORES, NSG, G, 2, M, W).transpose(0, 1, 3, 4, 2, 5)
    in_maps = [
        {"x": np.ascontiguousarray(xs[i]), "wm": wm} for i in range(N_CORES)
    ]
    res = run_bass_kernel_spmd(
        _get_nc(w_l, w_h), in_maps, core_ids=list(range(N_CORES)), **run_kwargs
    )
    # gather + inverse relayout: [NSG, half, p, o, g, c] -> [4, IMG, H, W]
    full = np.stack([r["out"] for r in res.results], axis=0)
    # full: [core, NSG, 2, M, 4, G, W] -> [o, core, NSG, G, 2, M, W]
    full = full.transpose(4, 0, 1, 5, 2, 3, 6).reshape(4, 8, 64, H, W)
    if run_kwargs:
        kernel.last_result = res  # expose profile info to test harnesses
    return (full[0], full[1], full[2], full[3])


# revision 22
# speedup vs baseline: 2.6045x; 1.0042x over previous
"""2D DWT (2-tap FFT reference) Trainium2 kernel.

The reference's FFT pipeline (pad to 256, circular conv, crop) reduces
algebraically to a 2x2 stencil per output:

    col pass:  v[r, c]   = wc1 * x[r, c] + wc0 * x[r+1, c]   (zero-ext r=224)
    row pass:  out[r, c] = wr1 * v[r, c] + wr0 * v[r, c+1]   (zero-ext c=224)

with (wc, wr) in {w_l, w_h}^2 for the four outputs: ll = (col l, row l),
lh = (col h, row l), hl = (col l, row h), hh = (col h, row h).

Kernel strategy (per core, 64 of the 512 independent images):
  * column pass on the tensor engine: v = S.T @ X with a banded stationary
    matrix S[p, m] = wc1*d(p==m) + wc0*d(p==m+1); image rows in SBUF
    partitions, two 112-row blocks per image; 16 images packed contiguously
    along the free dim (no pad columns), matmul windows of 2 images (448).
  * row pass fused into the PSUM drain: bounce v to SBUF (scalar engine),
    then out = beta*v + gamma*v_shift as one scalar_tensor_tensor per
    output over a [112, 2, 223] view (per-image, so no cross-image leak),
    plus a tiny per-image boundary column op (out[:,223] = beta*v[:,223]).
    beta is +-1 for Haar-type filters (S is pre-scaled by wl1); otherwise
    a premultiply fallback is used.
  * all DRAM tensors use a custom per-core layout [.., half, p, g, c] so
    every DMA descriptor is one fully contiguous 14336-byte run per
    partition (the host pre/post-transposes in numpy); input DMAs go via
    the gpsimd SWDGE queue, output DMAs via the sync HWDGE ring, so they
    overlap.
"""

import sys

for _p in ("/opt/trn_rl_repo", "/root/.axon_site/_ro/trn_rl_repo"):
    if _p not in sys.path:
        sys.path.append(_p)

import numpy as np

import concourse.bass as bass
import concourse.bacc as bacc
import concourse.mybir as mybir
from concourse import tile
from concourse.bass_utils import run_bass_kernel_spmd

N_CORES = 8
IMG = 64          # images per core  (512 total = 8 batch * 64 channels)
H = 224
W = 224
G = 8             # images per supertile
NSG = IMG // G    # supertile image-groups per core
FREE = G * W      # 1792 (contiguous, no pads)
SUBS = G // 2     # matmul windows per supertile (2 images each)
N = 2 * W         # 448 moving cols per window (<=512 fp32 limit)
M = 112           # output rows per matmul == half image height


def _row_coeffs(w_l, w_h):
    """Per-output (beta, gamma) for out = beta*v + gamma*v_sh, after the
    column matrices are scaled by alpha (so ll/lh get beta == 1)."""
    wl0, wl1 = float(w_l[0]), float(w_l[1])
    wh0, wh1 = float(w_h[0]), float(w_h[1])
    alpha = wl1 if abs(wl1) > 1e-30 else 1.0
    coeffs = [
        (wl1 / alpha, wl0 / alpha),   # ll: col l, row l
        (wl1 / alpha, wl0 / alpha),   # lh: col h, row l
        (wh1 / alpha, wh0 / alpha),   # hl: col l, row h
        (wh1 / alpha, wh0 / alpha),   # hh: col h, row h
    ]
    return alpha, coeffs


def _build_wmats(w_l, w_h):
    """Column-pass stationary matrices scaled by alpha, laid out
    [113, 4*112]: slot j = half*2 + f, f in {0: low, 1: high}."""
    alpha, _ = _row_coeffs(w_l, w_h)
    wm = np.zeros((113, 4 * M), np.float64)
    for half in range(2):
        K = 113 if half == 0 else 112
        for f, wc in enumerate([w_l, w_h]):
            S = np.zeros((113, M), np.float64)
            for m in range(M):
                S[m, m] = float(wc[1]) * alpha
                if m + 1 < K:
                    S[m + 1, m] = float(wc[0]) * alpha
            j = half * 2 + f
            wm[:, j * M:(j + 1) * M] = S
    return wm.astype(np.float32)


def _build_nc(beta_gamma) -> bass.Bass:
    """beta_gamma: list of 4 (beta, gamma) pairs baked as immediates."""
    nc = bacc.Bacc(
        "TRN2",
        target_bir_lowering=False,
        debug=False,
        num_devices=N_CORES,
    )
    f32 = mybir.dt.float32
    a = mybir.AluOpType
    # custom layouts: one contiguous (g, c) run per partition per DMA
    x = nc.dram_tensor("x", [NSG, 2, M, G, W], f32, kind="ExternalInput")
    wm = nc.dram_tensor("wm", [113, 4 * M], f32, kind="ExternalInput")
    out = nc.dram_tensor("out", [NSG, 2, M, 4, G, W], f32, kind="ExternalOutput")

    with tile.TileContext(nc) as tc:
        with (
            tc.tile_pool(name="wpool", bufs=1) as wpool,
            tc.tile_pool(name="xpool", bufs=4) as xpool,
            tc.tile_pool(name="opool", bufs=3) as opool,
            tc.tile_pool(name="tpool", bufs=3) as tpool,
            tc.tile_pool(name="pspool", bufs=3, space="PSUM") as pspool,
        ):
            wt = wpool.tile([113, 4 * M], f32)
            nc.gpsimd.dma_start(out=wt[0:112, :], in_=wm[0:112, :])
            nc.gpsimd.dma_start(out=wt[112:113, :], in_=wm[112:113, :])

            # persistent SBUF bounce buffers with a zero pad column per
            # image (stride 225) so the row-pass shift reads zero at the
            # image boundary; pads are zeroed once, manually rotated x3
            NBUF = 3
            svbufs = []
            for f in range(2):
                row = []
                for k in range(NBUF):
                    b = wpool.tile(
                        [M, 2 * (W + 1)], f32, tag=f"svb{f}_{k}",
                        name=f"svb{f}_{k}",
                    )
                    nc.vector.memset(
                        b[:, :].rearrange("p (i c) -> p i c", i=2)[:, :, W:W + 1],
                        0.0,
                    )
                    row.append(b)
                svbufs.append(row)

            for st in range(NSG * 2):
                sg, half = st // 2, st % 2
                K = 113 if half == 0 else 112

                xt = xpool.tile([113, FREE], f32, tag="xt", name=f"xt_{st}")
                nc.gpsimd.dma_start(
                    out=xt[0:112, :],
                    in_=x[sg, half].rearrange("p g c -> p (g c)"),
                )
                if K == 113:
                    # row 112 of this block == row 0 of the next half-block
                    nc.gpsimd.dma_start(
                        out=xt[112:113, :],
                        in_=x[sg, 1, 0:1].rearrange("p g c -> p (g c)"),
                    )

                otall = opool.tile(
                    [M, 4 * FREE], f32, tag="otall", name=f"otall_{st}"
                )
                for sub in range(SUBS):
                    s = sub * N
                    widx = st * SUBS + sub
                    svs = []
                    for f in range(2):  # 0: col-low, 1: col-high
                        v = pspool.tile(
                            [M, N], f32, tag=f"v{f}", name=f"v{f}_{st}_{sub}"
                        )
                        j0 = (half * 2 + f) * M
                        nc.tensor.matmul(
                            v[:, :],
                            wt[0:K, j0:j0 + M],
                            xt[0:K, s:s + N],
                            start=True,
                            stop=True,
                        )
                        # bounce to SBUF (one PSUM operand max per elemwise
                        # op); write only real cols, pads stay zero
                        sv = svbufs[f][widx % NBUF]
                        sv2 = sv[:, :].rearrange("p (i c) -> p i c", i=2)
                        nc.scalar.copy(
                            sv2[:, :, 0:W],
                            v[:, :].rearrange("p (i c) -> p i c", i=2),
                        )
                        svs.append(sv2)
                    # row pass over full 224-col views; the shifted read
                    # hits the zero pad at each image boundary
                    for o, sv2 in ((0, svs[0]), (1, svs[1]), (2, svs[0]), (3, svs[1])):
                        beta, gamma = beta_gamma[o]
                        ob = o * FREE + s
                        ot2 = otall[:, ob:ob + N].rearrange("p (i c) -> p i c", i=2)
                        sh = sv2[:, :, 1:W + 1]
                        base = sv2[:, :, 0:W]
                        plain = abs(gamma - 1.0) < 1e-12  # out = v_sh +- v
                        # keep Pool engine SWDGE-only (mixing Q7 compute with
                        # SWDGE descriptor generation hung intermittently)
                        eng = nc.vector
                        if plain and abs(beta - 1.0) < 1e-12:
                            eng.tensor_add(ot2[:, :, :], sh, base)
                        elif plain and abs(beta + 1.0) < 1e-12:
                            eng.tensor_sub(ot2[:, :, :], sh, base)
                        elif abs(beta - 1.0) < 1e-12:
                            eng.scalar_tensor_tensor(
                                ot2[:, :, :], sh, float(gamma), base,
                                a.mult, a.add,
                            )
                        elif abs(beta + 1.0) < 1e-12:
                            eng.scalar_tensor_tensor(
                                ot2[:, :, :], sh, float(gamma), base,
                                a.mult, a.subtract,
                            )
                        else:
                            tmp = tpool.tile(
                                [M, N], f32, tag="tmp", name=f"tmp{o}_{st}_{sub}"
                            )
                            tmp2 = tmp[:, :].rearrange("p (i c) -> p i c", i=2)
                            eng.tensor_scalar_mul(tmp2[:, :, :], base, float(beta))
                            eng.scalar_tensor_tensor(
                                ot2[:, :, :], sh, float(gamma), tmp2[:, :, :],
                                a.mult, a.add,
                            )

                nc.sync.dma_start(
                    out=out[sg, half].rearrange("p o g c -> p (o g c)"),
                    in_=otall[:, :],
                )
    nc.compile()
    return nc


_NC_CACHE: dict = {}


def _get_nc(w_l, w_h) -> bass.Bass:
    _, coeffs = _row_coeffs(w_l, w_h)
    key = tuple(coeffs[o] for o in range(4))
    if key not in _NC_CACHE:
        _NC_CACHE[key] = _build_nc(coeffs)
    return _NC_CACHE[key]


def kernel(x, w_l, w_h, **run_kwargs):
    x = np.asarray(x, dtype=np.float32)
    w_l = np.asarray(w_l, dtype=np.float32).reshape(-1)
    w_h = np.asarray(w_h, dtype=np.float32).reshape(-1)
    assert x.shape == (8, 64, H, W), x.shape
    assert w_l.shape == (2,) and w_h.shape == (2,)

    wm = _build_wmats(w_l, w_h)
    # per-core relayout: (IMG, 224, 224) -> (NSG, half, p, g, c)
    xs = x.reshape(N_CORES, NSG, G, 2, M, W).transpose(0, 1, 3, 4, 2, 5)
    in_maps = [
        {"x": np.ascontiguousarray(xs[i]), "wm": wm} for i in range(N_CORES)
    ]
    res = run_bass_kernel_spmd(
        _get_nc(w_l, w_h), in_maps, core_ids=list(range(N_CORES)), **run_kwargs
    )
    # gather + inverse relayout: [NSG, half, p, o, g, c] -> [4, IMG, H, W]
    full = np.stack([r["out"] for r in res.results], axis=0)
    # full: [core, NSG, 2, M, 4, G, W] -> [o, core, NSG, G, 2, M, W]
    full = full.transpose(4, 0, 1, 5, 2, 3, 6).reshape(4, 8, 64, H, W)
    if run_kwargs:
        kernel.last_result = res  # expose profile info to test harnesses
    return (full[0], full[1], full[2], full[3])


# revision 23
# speedup vs baseline: 2.6497x; 1.0174x over previous
"""2D DWT (2-tap FFT reference) Trainium2 kernel.

The reference's FFT pipeline (pad to 256, circular conv, crop) reduces
algebraically to a 2x2 stencil per output:

    col pass:  v[r, c]   = wc1 * x[r, c] + wc0 * x[r+1, c]   (zero-ext r=224)
    row pass:  out[r, c] = wr1 * v[r, c] + wr0 * v[r, c+1]   (zero-ext c=224)

with (wc, wr) in {w_l, w_h}^2 for the four outputs: ll = (col l, row l),
lh = (col h, row l), hl = (col l, row h), hh = (col h, row h).

Kernel strategy (per core, 64 of the 512 independent images):
  * column pass on the tensor engine: v = S.T @ X with a banded stationary
    matrix S[p, m] = wc1*d(p==m) + wc0*d(p==m+1); image rows in SBUF
    partitions, two 112-row blocks per image; 16 images packed contiguously
    along the free dim (no pad columns), matmul windows of 2 images (448).
  * row pass fused into the PSUM drain: bounce v to persistent SBUF
    buffers that carry a zero pad column per image (stride 225), then
    out = beta*v + gamma*v_shift as ONE vector op per output over a
    [112, 2, 224] view - the shifted read lands on the zero pad at each
    image boundary, so no separate boundary handling. beta is +-1 for
    Haar-type filters (S is pre-scaled by wl1), giving plain add/sub;
    otherwise scalar_tensor_tensor / premultiply fallbacks are used.
  * all DRAM tensors use a custom per-core layout [.., half, p, g, c] so
    every DMA descriptor is one fully contiguous 14336-byte run per
    partition (the host pre/post-transposes in numpy); input DMAs go via
    the gpsimd SWDGE queue, output DMAs via the sync HWDGE ring, so they
    overlap.
"""

import sys

for _p in ("/opt/trn_rl_repo", "/root/.axon_site/_ro/trn_rl_repo"):
    if _p not in sys.path:
        sys.path.append(_p)

import numpy as np

import concourse.bass as bass
import concourse.bacc as bacc
import concourse.mybir as mybir
from concourse import tile
from concourse.bass_utils import run_bass_kernel_spmd

N_CORES = 8
IMG = 64          # images per core  (512 total = 8 batch * 64 channels)
H = 224
W = 224
G = 8             # images per supertile
NSG = IMG // G    # supertile image-groups per core
FREE = G * W      # 1792 (contiguous, no pads)
SUBS = G // 2     # matmul windows per supertile (2 images each)
N = 2 * W         # 448 moving cols per window (<=512 fp32 limit)
M = 112           # output rows per matmul == half image height


def _row_coeffs(w_l, w_h):
    """Per-output (beta, gamma) for out = beta*v + gamma*v_sh, after the
    column matrices are scaled by alpha (so ll/lh get beta == 1)."""
    wl0, wl1 = float(w_l[0]), float(w_l[1])
    wh0, wh1 = float(w_h[0]), float(w_h[1])
    alpha = wl1 if abs(wl1) > 1e-30 else 1.0
    coeffs = [
        (wl1 / alpha, wl0 / alpha),   # ll: col l, row l
        (wl1 / alpha, wl0 / alpha),   # lh: col h, row l
        (wh1 / alpha, wh0 / alpha),   # hl: col l, row h
        (wh1 / alpha, wh0 / alpha),   # hh: col h, row h
    ]
    return alpha, coeffs


def _build_wmats(w_l, w_h):
    """Column-pass stationary matrices scaled by alpha, laid out
    [113, 4*112]: slot j = half*2 + f, f in {0: low, 1: high}."""
    alpha, _ = _row_coeffs(w_l, w_h)
    wm = np.zeros((113, 4 * M), np.float64)
    for half in range(2):
        K = 113 if half == 0 else 112
        for f, wc in enumerate([w_l, w_h]):
            S = np.zeros((113, M), np.float64)
            for m in range(M):
                S[m, m] = float(wc[1]) * alpha
                if m + 1 < K:
                    S[m + 1, m] = float(wc[0]) * alpha
            j = half * 2 + f
            wm[:, j * M:(j + 1) * M] = S
    return wm.astype(np.float32)


def _build_nc(beta_gamma) -> bass.Bass:
    """beta_gamma: list of 4 (beta, gamma) pairs baked as immediates."""
    nc = bacc.Bacc(
        "TRN2",
        target_bir_lowering=False,
        debug=False,
        num_devices=N_CORES,
    )
    f32 = mybir.dt.float32
    a = mybir.AluOpType
    # custom layouts: one contiguous (g, c) run per partition per DMA
    x = nc.dram_tensor("x", [NSG, 2, M, G, W], f32, kind="ExternalInput")
    wm = nc.dram_tensor("wm", [113, 4 * M], f32, kind="ExternalInput")
    out = nc.dram_tensor("out", [NSG, 2, M, 4, G, W], f32, kind="ExternalOutput")

    with tile.TileContext(nc) as tc:
        with (
            tc.tile_pool(name="wpool", bufs=1) as wpool,
            tc.tile_pool(name="xpool", bufs=4) as xpool,
            tc.tile_pool(name="opool", bufs=3) as opool,
            tc.tile_pool(name="tpool", bufs=3) as tpool,
            tc.tile_pool(name="pspool", bufs=3, space="PSUM") as pspool,
        ):
            wt = wpool.tile([113, 4 * M], f32)
            nc.gpsimd.dma_start(out=wt[0:112, :], in_=wm[0:112, :])
            nc.gpsimd.dma_start(out=wt[112:113, :], in_=wm[112:113, :])

            # persistent SBUF bounce buffers with a zero pad column per
            # image (stride 225) so the row-pass shift reads zero at the
            # image boundary; pads are zeroed once, manually rotated x3
            NBUF = 3
            svbufs = []
            for f in range(2):
                row = []
                for k in range(NBUF):
                    b = wpool.tile(
                        [M, 2 * (W + 1)], f32, tag=f"svb{f}_{k}",
                        name=f"svb{f}_{k}",
                    )
                    nc.vector.memset(
                        b[:, :].rearrange("p (i c) -> p i c", i=2)[:, :, W:W + 1],
                        0.0,
                    )
                    row.append(b)
                svbufs.append(row)

            for st in range(NSG * 2):
                sg, half = st // 2, st % 2
                K = 113 if half == 0 else 112

                xt = xpool.tile([113, FREE], f32, tag="xt", name=f"xt_{st}")
                nc.gpsimd.dma_start(
                    out=xt[0:112, :],
                    in_=x[sg, half].rearrange("p g c -> p (g c)"),
                )
                if K == 113:
                    # row 112 of this block == row 0 of the next half-block
                    nc.gpsimd.dma_start(
                        out=xt[112:113, :],
                        in_=x[sg, 1, 0:1].rearrange("p g c -> p (g c)"),
                    )

                otall = opool.tile(
                    [M, 4 * FREE], f32, tag="otall", name=f"otall_{st}"
                )
                for sub in range(SUBS):
                    s = sub * N
                    widx = st * SUBS + sub
                    svs = []
                    for f in range(2):  # 0: col-low, 1: col-high
                        v = pspool.tile(
                            [M, N], f32, tag=f"v{f}", name=f"v{f}_{st}_{sub}"
                        )
                        j0 = (half * 2 + f) * M
                        nc.tensor.matmul(
                            v[:, :],
                            wt[0:K, j0:j0 + M],
                            xt[0:K, s:s + N],
                            start=True,
                            stop=True,
                        )
                        # bounce to SBUF (one PSUM operand max per elemwise
                        # op); write only real cols, pads stay zero
                        sv = svbufs[f][widx % NBUF]
                        sv2 = sv[:, :].rearrange("p (i c) -> p i c", i=2)
                        nc.scalar.copy(
                            sv2[:, :, 0:W],
                            v[:, :].rearrange("p (i c) -> p i c", i=2),
                        )
                        svs.append(sv2)
                    # row pass over full 224-col views; the shifted read
                    # hits the zero pad at each image boundary
                    for o, sv2 in ((0, svs[0]), (1, svs[1]), (2, svs[0]), (3, svs[1])):
                        beta, gamma = beta_gamma[o]
                        ob = o * FREE + s
                        ot2 = otall[:, ob:ob + N].rearrange("p (i c) -> p i c", i=2)
                        sh = sv2[:, :, 1:W + 1]
                        base = sv2[:, :, 0:W]
                        plain = abs(gamma - 1.0) < 1e-12  # out = v_sh +- v
                        # keep Pool engine SWDGE-only (mixing Q7 compute with
                        # SWDGE descriptor generation hung intermittently)
                        eng = nc.vector
                        if plain and abs(beta - 1.0) < 1e-12:
                            eng.tensor_add(ot2[:, :, :], sh, base)
                        elif plain and abs(beta + 1.0) < 1e-12:
                            eng.tensor_sub(ot2[:, :, :], sh, base)
                        elif abs(beta - 1.0) < 1e-12:
                            eng.scalar_tensor_tensor(
                                ot2[:, :, :], sh, float(gamma), base,
                                a.mult, a.add,
                            )
                        elif abs(beta + 1.0) < 1e-12:
                            eng.scalar_tensor_tensor(
                                ot2[:, :, :], sh, float(gamma), base,
                                a.mult, a.subtract,
                            )
                        else:
                            tmp = tpool.tile(
                                [M, N], f32, tag="tmp", name=f"tmp{o}_{st}_{sub}"
                            )
                            tmp2 = tmp[:, :].rearrange("p (i c) -> p i c", i=2)
                            eng.tensor_scalar_mul(tmp2[:, :, :], base, float(beta))
                            eng.scalar_tensor_tensor(
                                ot2[:, :, :], sh, float(gamma), tmp2[:, :, :],
                                a.mult, a.add,
                            )

                nc.sync.dma_start(
                    out=out[sg, half].rearrange("p o g c -> p (o g c)"),
                    in_=otall[:, :],
                )
    nc.compile()
    return nc


_NC_CACHE: dict = {}


def _get_nc(w_l, w_h) -> bass.Bass:
    _, coeffs = _row_coeffs(w_l, w_h)
    key = tuple(coeffs[o] for o in range(4))
    if key not in _NC_CACHE:
        _NC_CACHE[key] = _build_nc(coeffs)
    return _NC_CACHE[key]


def kernel(x, w_l, w_h, **run_kwargs):
    x = np.asarray(x, dtype=np.float32)
    w_l = np.asarray(w_l, dtype=np.float32).reshape(-1)
    w_h = np.asarray(w_h, dtype=np.float32).reshape(-1)
    assert x.shape == (8, 64, H, W), x.shape
    assert w_l.shape == (2,) and w_h.shape == (2,)

    wm = _build_wmats(w_l, w_h)
    # per-core relayout: (IMG, 224, 224) -> (NSG, half, p, g, c)
    xs = x.reshape(N_CORES, NSG, G, 2, M, W).transpose(0, 1, 3, 4, 2, 5)
    in_maps = [
        {"x": np.ascontiguousarray(xs[i]), "wm": wm} for i in range(N_CORES)
    ]
    res = run_bass_kernel_spmd(
        _get_nc(w_l, w_h), in_maps, core_ids=list(range(N_CORES)), **run_kwargs
    )
    # gather + inverse relayout: [NSG, half, p, o, g, c] -> [4, IMG, H, W]
    full = np.stack([r["out"] for r in res.results], axis=0)
    # full: [core, NSG, 2, M, 4, G, W] -> [o, core, NSG, G, 2, M, W]
    full = full.transpose(4, 0, 1, 5, 2, 3, 6).reshape(4, 8, 64, H, W)
    if run_kwargs:
        kernel.last_result = res  # expose profile info to test harnesses
    return (full[0], full[1], full[2], full[3])
